# revision 47
# baseline (speedup 1.0000x reference)
"""Causal self-attention Trainium2 kernel (fused-pipeline version).

Shards batch(4) x head-group(2) across 8 NeuronCores. Each core computes, for
its batch b and its 8 heads:
    qkv = x[b] @ w_qkv_shard
    per head: S^T = k q^T / 8 (causal), P^T = exp(S^T), [o^T; den] = [v|1]^T P^T
    partial_out = o_all @ w_proj_shard
Host sums the two head-group partials per batch and adds b_proj.

Pipeline structure (single pass, no phase barriers):
  - prologue: qT/kT for head pair 0, then v for all heads (paced by the
    input DMAs, which are issued interleaved per contraction tile).
  - pair p attention runs with the qT/kT projection of pair p+1 (and, for
    the last pair, the output projection) emitted as PE "filler" between
    score groups, so the in-order PE never idles waiting on the scalar
    engine's exp and the HAM throttle stays at full rate.
  - scores / exp / AV / output-projection all skip the fully-masked region
    of diagonal blocks (causal narrowing).
  - per-chunk softmax denominators: reciprocal_approx_fast on a [1,512]
    row, partition_broadcast, one fused normalize multiply into oT.
All matmul operands are bf16; PSUM accumulation is fp32; the output is
returned bf16 and upcast host-side.
"""

import numpy as np
import ml_dtypes
from collections import deque
from contextlib import ExitStack

import concourse.bass as bass
import concourse.bacc as bacc
import concourse.mybir as mybir
import concourse.tile as tile
from concourse import bass_utils
from concourse.masks import make_upper_triangular

D = 1024
T = 2048
B = 4
NH = 16          # global heads
HD = 64
NCORES = 8
HL = 8           # heads per core (local)
DS = HL * HD     # 512: per-core head-feature width

F32 = mybir.dt.float32
BF16 = mybir.dt.bfloat16
DT = BF16
EXPF = mybir.ActivationFunctionType.Exp

TQ = 512         # tq chunk width (scores free dim)
NTT = T // 128   # 16 token tiles
NK = D // 128    # 8 contraction tiles
GRP = 2          # score tiles per exp batch
LAG = 5          # AV lag (score tiles) behind scores/exp


def _build(with_bias: bool):
    nc = bacc.Bacc("TRN2", target_bir_lowering=False, debug=False,
                   num_devices=NCORES)
    KROWS = D + 1 if with_bias else D
    xT = nc.dram_tensor("xT", [KROWS, T], DT, kind="ExternalInput")
    w = nc.dram_tensor("w", [KROWS, 3 * DS], DT, kind="ExternalInput")
    wp = nc.dram_tensor("wp", [DS, D], DT, kind="ExternalInput")
    out = nc.dram_tensor("out", [T, D], DT, kind="ExternalOutput")

    with tile.TileContext(nc) as tc, ExitStack() as ctx:
        big = ctx.enter_context(tc.tile_pool(name="big", bufs=1))
        pt_pool = ctx.enter_context(tc.tile_pool(name="pt", bufs=5))
        osb_pool = ctx.enter_context(tc.tile_pool(name="osb", bufs=3))
        den_pool = ctx.enter_context(tc.tile_pool(name="den", bufs=4))
        bcs_pool = ctx.enter_context(tc.tile_pool(name="bcs", bufs=2))
        cst_pool = ctx.enter_context(tc.tile_pool(name="cst", bufs=4))
        fps = ctx.enter_context(tc.tile_pool(name="fps", bufs=2,
                                             space="PSUM"))
        scp = ctx.enter_context(tc.tile_pool(name="scp", bufs=2,
                                             space="PSUM"))
        osp = ctx.enter_context(tc.tile_pool(name="osp", bufs=2,
                                             space="PSUM"))

        # ---- input loads ----
        # w is host-reordered to [q0k0|q1k1|q2k2|q3k3|v] so head pair 0's
        # projection weights arrive first; transfers are split per region so
        # the pair-0 matmuls aren't gated on the full 3MB weight load.
        # x is loaded as two half-tiles per contraction tile so the first
        # projection matmuls are gated on 262KB arrivals instead of 524KB
        xth = [[big.tile([128, T // 2], DT, tag=f"xt{h}_{k}",
                         name=f"xt{h}_{k}") for k in range(NK)]
               for h in range(2)]
        wqk = [[big.tile([128, 256], DT, tag=f"wqk{p}_{k}",
                         name=f"wqk{p}_{k}") for k in range(NK)]
               for p in range(4)]
        wv = [big.tile([128, DS], DT, tag=f"wv{k}", name=f"wv{k}")
              for k in range(NK)]
        for k in range(NK):
            nc.sync.dma_start(xth[0][k][:],
                              xT.ap()[k * 128:(k + 1) * 128, 0:T // 2])
            nc.sync.dma_start(wqk[0][k][:],
                              w.ap()[k * 128:(k + 1) * 128, 0:256])
        for k in range(NK):
            nc.sync.dma_start(xth[1][k][:],
                              xT.ap()[k * 128:(k + 1) * 128, T // 2:T])
        for k in range(NK):
            nc.sync.dma_start(wv[k][:],
                              w.ap()[k * 128:(k + 1) * 128, 2 * DS:3 * DS])
        for p in range(1, 4):
            for k in range(NK):
                nc.sync.dma_start(
                    wqk[p][k][:],
                    w.ap()[k * 128:(k + 1) * 128, 256 * p:256 * (p + 1)])
        if with_bias:
            xb = big.tile([1, T], DT, tag="xb", name="xb")
            nc.sync.dma_start(xb[:], xT.ap()[D:D + 1, :])
            wb = big.tile([1, 3 * DS], DT, tag="wb", name="wb")
            nc.sync.dma_start(wb[:], w.ap()[D:D + 1, :])
        wpt = []
        for k in range(DS // 128):
            t_ = big.tile([128, D], DT, tag=f"wpt{k}", name=f"wpt{k}")
            nc.sync.dma_start(t_[:], wp.ap()[k * 128:(k + 1) * 128, :])
            wpt.append(t_)

        # ---- PE warm-up ----
        # ~8 matmuls on a memset scratch bridge the PE from t=0 to the first
        # input-gated matmul so the HAM throttle ramps to full rate early.
        scr = big.tile([128, 512], DT, tag="scr", name="scr")
        nc.gpsimd.memset(scr[:], 0.0)
        wps = fps.tile([128, 512], F32, tag="fps", name="fps")
        for i in range(8):
            nc.tensor.matmul(wps[:], lhsT=scr[:, 0:128], rhs=scr[:],
                             start=True, stop=True)

        # ---- persistent SBUF tensors ----
        maskf = big.tile([128, 128], F32, tag="maskf", name="maskf")
        make_upper_triangular(nc, maskf[:], val=1.0, diag=True)
        mask = big.tile([128, 128], DT, tag="mask", name="mask")
        nc.vector.tensor_copy(mask[:], maskf[:])

        # q^T|k^T per head pair, SBUF-resident: m 0..3 = q pairs, 4..7 = k
        qkT = [big.tile([128, T], DT, tag=f"qkT{m}", name=f"qkT{m}")
               for m in range(8)]
        # v token-major, packed per head with a ones column (64 v + 1 one)
        v_ext = [big.tile([128, HL * (HD + 1) + 64], DT, tag=f"vext{t}",
                          name=f"vext{t}")
                 for t in range(NTT)]
        for t in range(NTT):
            ones_cols = v_ext[t][:, 0:HL * (HD + 1)].rearrange(
                "p (h c) -> p h c", h=HL)[:, :, HD:HD + 1]
            nc.gpsimd.memset(ones_cols, 1.0)
            # head 7's 128-wide lhsT window reaches into this pad
            nc.gpsimd.memset(v_ext[t][:, HL * (HD + 1):], 0.0)
        # normalized attention outputs, head-dim major
        oT = [big.tile([128, T], DT, tag=f"ot{m}", name=f"ot{m}")
              for m in range(4)]

        # ---- PE filler generators ----
        def gen_a(p):
            """qT/kT projection for head pair p; yields every matmul."""
            for n in range(T // 512):
                for qk in range(2):
                    m = p + 4 * qk
                    ps = fps.tile([128, 512], F32, tag="fps", name="fps")
                    for k in range(NK):
                        nc.tensor.matmul(
                            ps[:],
                            lhsT=wqk[p][k][:, qk * 128:(qk + 1) * 128],
                            rhs=xth[n // 2][k][:, (n % 2) * 512:
                                               (n % 2 + 1) * 512],
                            start=(k == 0),
                            stop=(k == NK - 1 and not with_bias))
                        yield
                    if with_bias:
                        nc.tensor.matmul(
                            ps[:],
                            lhsT=wb[0:1, 256 * p + 128 * qk:
                                  256 * p + 128 * (qk + 1)],
                            rhs=xb[0:1, n * 512:(n + 1) * 512],
                            start=False, stop=True)
                    nc.vector.tensor_copy(
                        qkT[m][:, n * 512:(n + 1) * 512], ps[:])
                    yield

        v_prog = [0]  # number of v token tiles fully emitted

        def gen_v():
            """v projection for all heads; yields every matmul."""
            for t in range(NTT):
                ps = fps.tile([128, 512], F32, tag="fps", name="fps")
                for k in range(NK):
                    nc.tensor.matmul(
                        ps[:],
                        lhsT=xth[t // 8][k][:, (t % 8) * 128:
                                            (t % 8 + 1) * 128],
                        rhs=wv[k][:],
                        start=(k == 0),
                        stop=(k == NK - 1 and not with_bias))
                    yield
                if with_bias:
                    nc.tensor.matmul(
                        ps[:],
                        lhsT=xb[0:1, t * 128:(t + 1) * 128],
                        rhs=wb[0:1, 2 * DS:3 * DS],
                        start=False, stop=True)
                dst = v_ext[t][:, 0:HL * (HD + 1)].rearrange(
                    "p (h c) -> p h c", h=HL)[:, :, 0:HD]
                src = ps.rearrange("p (h c) -> p h c", h=HL)
                # scalar engine is idle during the prologue v block; DVE
                # copies here stalled the psum pool recycling
                nc.scalar.copy(dst, src)
                v_prog[0] = t + 1
                yield

        def gen_c(c):
            """output projection for token tiles of chunk c."""
            for t in range(4 * c, 4 * c + 4):
                for n2 in range(2):
                    ps = fps.tile([128, 512], F32, tag="fps", name="fps")
                    for m in range(4):
                        nc.tensor.matmul(
                            ps[:],
                            lhsT=oT[m][:, t * 128:(t + 1) * 128],
                            rhs=wpt[m][:, n2 * 512:(n2 + 1) * 512],
                            start=(m == 0), stop=(m == 3))
                        yield
                    st = cst_pool.tile([128, 512], DT, tag="cst", name="cst")
                    # scalar engine is idle during the projection tail
                    nc.scalar.copy(st[:], ps[:])
                    nc.sync.dma_start(
                        out.ap()[t * 128:(t + 1) * 128,
                                 n2 * 512:(n2 + 1) * 512], st[:])
                    yield

        filler = deque()

        def pump(n):
            while n > 0 and filler:
                try:
                    next(filler[0])
                    n -= 1
                except StopIteration:
                    filler.popleft()

        # ---- prologue: pair-0 projections, then v ----
        # the first two psum groups are paced by the input DMAs (~1.6us per
        # contraction tile); pad with dummy matmuls so the PE never idles
        # long enough to reset the HAM throttle warm-up
        dps = osp.tile([128, 512], F32, tag="osp", name="osp")
        for i, _ in enumerate(gen_a(0)):
            if i < 8:
                for _ in range(2):
                    nc.tensor.matmul(dps[:], lhsT=scr[:, 0:128], rhs=scr[:],
                                     start=True, stop=True)
        for i, _ in enumerate(gen_v()):
            if i < 8:  # first v tile is paced by the wv arrivals
                nc.tensor.matmul(dps[:], lhsT=scr[:, 0:128], rhs=scr[:],
                                 start=True, stop=True)

        # ---- attention pairs, with filler interleave ----
        for p in range(4):
            # drain leftover filler first: it contains the tail of the
            # qT/kT projection this pair is about to read
            while filler:
                try:
                    next(filler[0])
                except StopIteration:
                    filler.popleft()
            if p < 3:
                filler.append(gen_a(p + 1))
            qt = qkT[p]
            kt = qkT[4 + p]
            pending = []

            def emit_av(item, p=p):
                hl, c, po, op, t, ntk, pt = item
                vs = max(0, t - 4 * c) * 128
                nc.tensor.matmul(
                    op[:, vs:512],
                    lhsT=v_ext[t][:, hl * (HD + 1):hl * (HD + 1) + 128],
                    rhs=pt[:, vs:512],
                    start=(t == 0), stop=(t == ntk - 1))
                if t == ntk - 1:
                    # evacuate o rows, then per-chunk normalize
                    o_sb = osb_pool.tile([HD, 512], DT, tag="osb",
                                         name="osb")
                    nc.vector.tensor_copy(o_sb[:], op[0:HD, :])
                    # NOTE: reciprocal_approx_fast must read SBUF — feeding
                    # it PSUM directly passes CoreSim but corrupts on HW
                    dsb = den_pool.tile([1, 512], F32, tag="dsb", name="dsb")
                    nc.vector.tensor_copy(dsb[:], op[HD:HD + 1, :])
                    rc = den_pool.tile([1, 512], F32, tag="rc", name="rc")
                    nc.vector.reciprocal_approx_fast(out=rc[:], in_=dsb[:])
                    bcs = bcs_pool.tile([HD, 512], F32, tag="bcs",
                                        name="bcs")
                    nc.gpsimd.partition_broadcast(bcs[:], rc[:])
                    nc.vector.tensor_mul(
                        oT[p][po:po + HD, c * TQ:(c + 1) * TQ],
                        o_sb[:], bcs[:])
                    if p == 3 and po == 64:
                        # chunk c of the last pair fully normalized: its
                        # output projection becomes available as filler
                        filler.append(gen_c(c))

            # pair 3 runs its chunks largest-first: chunk 3's output
            # projection then feeds the starved small chunks as filler
            for c in (range(T // TQ) if p < 3 else (3, 2, 1, 0)):
                ntk = 4 * c + 4
                for h01 in range(2):
                    hl = 2 * p + h01
                    po = h01 * 64
                    op = osp.tile([128, 512], F32, tag="osp", name="osp")
                    for t0g in range(0, ntk, GRP):
                        ps = scp.tile([128, GRP * 512], F32, tag="scp",
                                      name="scp")
                        vss = []
                        for s in range(GRP):
                            t = t0g + s
                            vs = max(0, t - 4 * c) * 128
                            vss.append(vs)
                            nc.tensor.matmul(
                                ps[:, s * 512 + vs:(s + 1) * 512],
                                lhsT=kt[po:po + 64,
                                        t * 128:(t + 1) * 128],
                                rhs=qt[po:po + 64,
                                       c * TQ + vs:(c + 1) * TQ],
                                start=True, stop=True)
                        pt = pt_pool.tile([128, GRP * 512], DT,
                                          tag="pt", name="pt")
                        # exp over the valid (non-fully-masked) region only
                        if vss[1] == 0:
                            nc.scalar.activation(
                                pt[:, vss[0]:GRP * 512],
                                ps[:, vss[0]:GRP * 512], EXPF, scale=0.125)
                        else:
                            nc.scalar.activation(
                                pt[:, vss[0]:512],
                                ps[:, vss[0]:512], EXPF, scale=0.125)
                            nc.scalar.activation(
                                pt[:, 512 + vss[1]:1024],
                                ps[:, 512 + vss[1]:1024], EXPF, scale=0.125)
                        for s in range(GRP):
                            t = t0g + s
                            j = t - 4 * c
                            if j >= 0:  # mask the diagonal 128x128 block
                                blk = pt[:, s * 512 + j * 128:
                                         s * 512 + (j + 1) * 128]
                                nc.vector.tensor_mul(blk, blk, mask[:])
                        # early chunks have little PE work per exp group, so
                        # feed them more filler to keep the PE streaming
                        pump((3, 2, 2, 1)[c] if p < 2 else
                             ((3, 2, 2, 2)[c] if p == 2
                              else (5, 3, 3, 1)[c]))
                        for s in range(GRP):
                            t = t0g + s
                            if len(pending) >= LAG:
                                emit_av(pending.pop(0))
                            pending.append((hl, c, po, op, t, ntk,
                                            pt[:, s * 512:(s + 1) * 512]))
            while pending:
                emit_av(pending.pop(0))
                # keep the PE fed while the drain ladder waits on exp
                pump(3)
        # drain remaining filler (tail of the output projection)
        while filler:
            try:
                next(filler[0])
            except StopIteration:
                filler.popleft()

    nc.compile()
    return nc


_CACHE = {}


def _get_nc(with_bias: bool):
    if with_bias not in _CACHE:
        _CACHE[with_bias] = _build(with_bias)
    return _CACHE[with_bias]


def make_in_maps(x, w_qkv, b_qkv, w_proj, with_bias):
    """Per-core input dicts (host-side shard + transpose + bf16 cast)."""
    x = np.asarray(x, dtype=np.float32)
    w_qkv = np.asarray(w_qkv, dtype=np.float32)
    b_qkv = np.asarray(b_qkv, dtype=np.float32)
    w_proj = np.asarray(w_proj, dtype=np.float32)
    cast = lambda a: np.ascontiguousarray(a).astype(ml_dtypes.bfloat16)
    in_maps = []
    for core in range(NCORES):
        b, hg = divmod(core, 2)
        # w columns ordered [q0k0|q1k1|q2k2|q3k3|v]: per head pair p, its
        # 128 q columns then its 128 k columns; the v block unchanged
        qk_cols = []
        for p in range(4):
            qk_cols.append(np.r_[hg * DS + 128 * p:hg * DS + 128 * (p + 1)])
            qk_cols.append(np.r_[D + hg * DS + 128 * p:
                                 D + hg * DS + 128 * (p + 1)])
        qk_cols.append(np.arange(2 * D + hg * DS, 2 * D + hg * DS + DS))
        cols = np.concatenate(qk_cols)
        w_s = w_qkv[:, cols]                      # [D, 3*DS]
        xTa = np.ascontiguousarray(x[b].T)        # [D, T]
        if with_bias:
            xTa = np.concatenate([xTa, np.ones((1, T), np.float32)], axis=0)
            w_s = np.concatenate([w_s, b_qkv[cols][None, :]], axis=0)
        in_maps.append({
            "xT": cast(xTa),
            "w": cast(w_s),
            "wp": cast(w_proj[hg * DS:(hg + 1) * DS, :]),
        })
    return in_maps


LAST_EXEC_TIME_NS = None


def kernel(x, w_qkv, b_qkv, w_proj, b_proj):
    global LAST_EXEC_TIME_NS
    with_bias = bool(np.any(np.asarray(b_qkv)))
    nc = _get_nc(with_bias)
    in_maps = make_in_maps(x, w_qkv, b_qkv, w_proj, with_bias)
    res = bass_utils.run_bass_kernel_spmd(
        nc, in_maps, core_ids=list(range(NCORES)))
    LAST_EXEC_TIME_NS = res.exec_time_ns
    b_proj = np.asarray(b_proj, dtype=np.float32)
    out = np.empty((B, T, D), dtype=np.float32)
    for b in range(B):
        out[b] = (np.asarray(res.results[2 * b]["out"], dtype=np.float32)
                  + np.asarray(res.results[2 * b + 1]["out"],
                               dtype=np.float32)
                  + b_proj)
    return out


# revision 48
# speedup vs baseline: 1.0083x; 1.0083x over previous
"""Causal self-attention Trainium2 kernel (fused-pipeline version).

Shards batch(4) x head-group(2) across 8 NeuronCores. Each core computes, for
its batch b and its 8 heads:
    qkv = x[b] @ w_qkv_shard
    per head: S^T = k q^T / 8 (causal), P^T = exp(S^T), [o^T; den] = [v|1]^T P^T
    partial_out = o_all @ w_proj_shard
Host sums the two head-group partials per batch and adds b_proj.

Pipeline structure (single pass, no phase barriers):
  - prologue: qT/kT for head pair 0, then v for all heads (paced by the
    input DMAs, which are issued interleaved per contraction tile).
  - pair p attention runs with the qT/kT projection of pair p+1 (and, for
    the last pair, the output projection) emitted as PE "filler" between
    score groups, so the in-order PE never idles waiting on the scalar
    engine's exp and the HAM throttle stays at full rate.
  - scores / exp / AV / output-projection all skip the fully-masked region
    of diagonal blocks (causal narrowing).
  - per-chunk softmax denominators: reciprocal_approx_fast on a [1,512]
    row, partition_broadcast, one fused normalize multiply into oT.
All matmul operands are bf16; PSUM accumulation is fp32; the output is
returned bf16 and upcast host-side.
"""

import numpy as np
import ml_dtypes
from collections import deque
from contextlib import ExitStack

import concourse.bass as bass
import concourse.bacc as bacc
import concourse.mybir as mybir
import concourse.tile as tile
from concourse import bass_utils
from concourse.masks import make_upper_triangular

D = 1024
T = 2048
B = 4
NH = 16          # global heads
HD = 64
NCORES = 8
HL = 8           # heads per core (local)
DS = HL * HD     # 512: per-core head-feature width

F32 = mybir.dt.float32
BF16 = mybir.dt.bfloat16
DT = BF16
EXPF = mybir.ActivationFunctionType.Exp

TQ = 512         # tq chunk width (scores free dim)
NTT = T // 128   # 16 token tiles
NK = D // 128    # 8 contraction tiles
GRP = 2          # score tiles per exp batch
LAG = 5          # AV lag (score tiles) behind scores/exp


def _build(with_bias: bool):
    nc = bacc.Bacc("TRN2", target_bir_lowering=False, debug=False,
                   num_devices=NCORES)
    KROWS = D + 1 if with_bias else D
    xT = nc.dram_tensor("xT", [KROWS, T], DT, kind="ExternalInput")
    w = nc.dram_tensor("w", [KROWS, 3 * DS], DT, kind="ExternalInput")
    wp = nc.dram_tensor("wp", [DS, D], DT, kind="ExternalInput")
    out = nc.dram_tensor("out", [T, D], DT, kind="ExternalOutput")

    with tile.TileContext(nc) as tc, ExitStack() as ctx:
        big = ctx.enter_context(tc.tile_pool(name="big", bufs=1))
        pt_pool = ctx.enter_context(tc.tile_pool(name="pt", bufs=5))
        osb_pool = ctx.enter_context(tc.tile_pool(name="osb", bufs=4))
        den_pool = ctx.enter_context(tc.tile_pool(name="den", bufs=4))
        bcs_pool = ctx.enter_context(tc.tile_pool(name="bcs", bufs=2))
        cst_pool = ctx.enter_context(tc.tile_pool(name="cst", bufs=6))
        fps = ctx.enter_context(tc.tile_pool(name="fps", bufs=2,
                                             space="PSUM"))
        scp = ctx.enter_context(tc.tile_pool(name="scp", bufs=2,
                                             space="PSUM"))
        osp = ctx.enter_context(tc.tile_pool(name="osp", bufs=2,
                                             space="PSUM"))

        # ---- input loads ----
        # w is host-reordered to [q0k0|q1k1|q2k2|q3k3|v] so head pair 0's
        # projection weights arrive first; transfers are split per region so
        # the pair-0 matmuls aren't gated on the full 3MB weight load.
        # x is loaded as two half-tiles per contraction tile so the first
        # projection matmuls are gated on 262KB arrivals instead of 524KB
        xth = [[big.tile([128, T // 2], DT, tag=f"xt{h}_{k}",
                         name=f"xt{h}_{k}") for k in range(NK)]
               for h in range(2)]
        wqk = [[big.tile([128, 256], DT, tag=f"wqk{p}_{k}",
                         name=f"wqk{p}_{k}") for k in range(NK)]
               for p in range(4)]
        wv = [big.tile([128, DS], DT, tag=f"wv{k}", name=f"wv{k}")
              for k in range(NK)]
        for k in range(NK):
            nc.sync.dma_start(xth[0][k][:],
                              xT.ap()[k * 128:(k + 1) * 128, 0:T // 2])
            nc.sync.dma_start(wqk[0][k][:],
                              w.ap()[k * 128:(k + 1) * 128, 0:256])
        for k in range(NK):
            nc.sync.dma_start(xth[1][k][:],
                              xT.ap()[k * 128:(k + 1) * 128, T // 2:T])
        for k in range(NK):
            nc.sync.dma_start(wv[k][:],
                              w.ap()[k * 128:(k + 1) * 128, 2 * DS:3 * DS])
        for p in range(1, 4):
            for k in range(NK):
                nc.sync.dma_start(
                    wqk[p][k][:],
                    w.ap()[k * 128:(k + 1) * 128, 256 * p:256 * (p + 1)])
        if with_bias:
            xb = big.tile([1, T], DT, tag="xb", name="xb")
            nc.sync.dma_start(xb[:], xT.ap()[D:D + 1, :])
            wb = big.tile([1, 3 * DS], DT, tag="wb", name="wb")
            nc.sync.dma_start(wb[:], w.ap()[D:D + 1, :])
        wpt = []
        for k in range(DS // 128):
            t_ = big.tile([128, D], DT, tag=f"wpt{k}", name=f"wpt{k}")
            nc.sync.dma_start(t_[:], wp.ap()[k * 128:(k + 1) * 128, :])
            wpt.append(t_)

        # ---- PE warm-up ----
        # ~8 matmuls on a memset scratch bridge the PE from t=0 to the first
        # input-gated matmul so the HAM throttle ramps to full rate early.
        scr = big.tile([128, 512], DT, tag="scr", name="scr")
        nc.gpsimd.memset(scr[:], 0.0)
        wps = fps.tile([128, 512], F32, tag="fps", name="fps")
        for i in range(8):
            nc.tensor.matmul(wps[:], lhsT=scr[:, 0:128], rhs=scr[:],
                             start=True, stop=True)

        # ---- persistent SBUF tensors ----
        maskf = big.tile([128, 128], F32, tag="maskf", name="maskf")
        make_upper_triangular(nc, maskf[:], val=1.0, diag=True)
        mask = big.tile([128, 128], DT, tag="mask", name="mask")
        nc.vector.tensor_copy(mask[:], maskf[:])

        # q^T|k^T per head pair, SBUF-resident: m 0..3 = q pairs, 4..7 = k
        qkT = [big.tile([128, T], DT, tag=f"qkT{m}", name=f"qkT{m}")
               for m in range(8)]
        # v token-major, packed per head with a ones column (64 v + 1 one)
        v_ext = [big.tile([128, HL * (HD + 1) + 64], DT, tag=f"vext{t}",
                          name=f"vext{t}")
                 for t in range(NTT)]
        for t in range(NTT):
            ones_cols = v_ext[t][:, 0:HL * (HD + 1)].rearrange(
                "p (h c) -> p h c", h=HL)[:, :, HD:HD + 1]
            nc.gpsimd.memset(ones_cols, 1.0)
            # head 7's 128-wide lhsT window reaches into this pad
            nc.gpsimd.memset(v_ext[t][:, HL * (HD + 1):], 0.0)
        # normalized attention outputs, head-dim major
        oT = [big.tile([128, T], DT, tag=f"ot{m}", name=f"ot{m}")
              for m in range(4)]

        # ---- PE filler generators ----
        def gen_a(p):
            """qT/kT projection for head pair p; yields every matmul."""
            for n in range(T // 512):
                for qk in range(2):
                    m = p + 4 * qk
                    ps = fps.tile([128, 512], F32, tag="fps", name="fps")
                    for k in range(NK):
                        nc.tensor.matmul(
                            ps[:],
                            lhsT=wqk[p][k][:, qk * 128:(qk + 1) * 128],
                            rhs=xth[n // 2][k][:, (n % 2) * 512:
                                               (n % 2 + 1) * 512],
                            start=(k == 0),
                            stop=(k == NK - 1 and not with_bias))
                        yield
                    if with_bias:
                        nc.tensor.matmul(
                            ps[:],
                            lhsT=wb[0:1, 256 * p + 128 * qk:
                                  256 * p + 128 * (qk + 1)],
                            rhs=xb[0:1, n * 512:(n + 1) * 512],
                            start=False, stop=True)
                    nc.vector.tensor_copy(
                        qkT[m][:, n * 512:(n + 1) * 512], ps[:])
                    yield

        v_prog = [0]  # number of v token tiles fully emitted

        def gen_v():
            """v projection for all heads; yields every matmul."""
            for t in range(NTT):
                ps = fps.tile([128, 512], F32, tag="fps", name="fps")
                for k in range(NK):
                    nc.tensor.matmul(
                        ps[:],
                        lhsT=xth[t // 8][k][:, (t % 8) * 128:
                                            (t % 8 + 1) * 128],
                        rhs=wv[k][:],
                        start=(k == 0),
                        stop=(k == NK - 1 and not with_bias))
                    yield
                if with_bias:
                    nc.tensor.matmul(
                        ps[:],
                        lhsT=xb[0:1, t * 128:(t + 1) * 128],
                        rhs=wb[0:1, 2 * DS:3 * DS],
                        start=False, stop=True)
                dst = v_ext[t][:, 0:HL * (HD + 1)].rearrange(
                    "p (h c) -> p h c", h=HL)[:, :, 0:HD]
                src = ps.rearrange("p (h c) -> p h c", h=HL)
                # scalar engine is idle during the prologue v block; DVE
                # copies here stalled the psum pool recycling
                nc.scalar.copy(dst, src)
                v_prog[0] = t + 1
                yield

        def gen_c(c):
            """output projection for token tiles of chunk c."""
            for t in range(4 * c, 4 * c + 4):
                for n2 in range(2):
                    ps = fps.tile([128, 512], F32, tag="fps", name="fps")
                    for m in range(4):
                        nc.tensor.matmul(
                            ps[:],
                            lhsT=oT[m][:, t * 128:(t + 1) * 128],
                            rhs=wpt[m][:, n2 * 512:(n2 + 1) * 512],
                            start=(m == 0), stop=(m == 3))
                        yield
                    st = cst_pool.tile([128, 512], DT, tag="cst", name="cst")
                    # scalar engine is idle during the projection tail
                    nc.scalar.copy(st[:], ps[:])
                    nc.sync.dma_start(
                        out.ap()[t * 128:(t + 1) * 128,
                                 n2 * 512:(n2 + 1) * 512], st[:])
                    yield

        filler = deque()

        def pump(n):
            while n > 0 and filler:
                try:
                    next(filler[0])
                    n -= 1
                except StopIteration:
                    filler.popleft()

        # ---- prologue: pair-0 projections, then v ----
        # the first two psum groups are paced by the input DMAs (~1.6us per
        # contraction tile); pad with dummy matmuls so the PE never idles
        # long enough to reset the HAM throttle warm-up
        dps = osp.tile([128, 512], F32, tag="osp", name="osp")
        for i, _ in enumerate(gen_a(0)):
            if i < 8:
                for _ in range(2):
                    nc.tensor.matmul(dps[:], lhsT=scr[:, 0:128], rhs=scr[:],
                                     start=True, stop=True)
        for i, _ in enumerate(gen_v()):
            if i < 8:  # first v tile is paced by the wv arrivals
                nc.tensor.matmul(dps[:], lhsT=scr[:, 0:128], rhs=scr[:],
                                 start=True, stop=True)

        # ---- attention pairs, with filler interleave ----
        for p in range(4):
            # drain leftover filler first: it contains the tail of the
            # qT/kT projection this pair is about to read
            while filler:
                try:
                    next(filler[0])
                except StopIteration:
                    filler.popleft()
            if p < 3:
                filler.append(gen_a(p + 1))
            qt = qkT[p]
            kt = qkT[4 + p]
            pending = []

            def emit_av(item, p=p):
                hl, c, po, op, t, ntk, pt = item
                vs = max(0, t - 4 * c) * 128
                nc.tensor.matmul(
                    op[:, vs:512],
                    lhsT=v_ext[t][:, hl * (HD + 1):hl * (HD + 1) + 128],
                    rhs=pt[:, vs:512],
                    start=(t == 0), stop=(t == ntk - 1))
                if t == ntk - 1:
                    # evacuate o rows, then per-chunk normalize
                    o_sb = osb_pool.tile([HD, 512], DT, tag="osb",
                                         name="osb")
                    nc.vector.tensor_copy(o_sb[:], op[0:HD, :])
                    # NOTE: reciprocal_approx_fast must read SBUF — feeding
                    # it PSUM directly passes CoreSim but corrupts on HW
                    dsb = den_pool.tile([1, 512], F32, tag="dsb", name="dsb")
                    nc.vector.tensor_copy(dsb[:], op[HD:HD + 1, :])
                    rc = den_pool.tile([1, 512], F32, tag="rc", name="rc")
                    nc.vector.reciprocal_approx_fast(out=rc[:], in_=dsb[:])
                    bcs = bcs_pool.tile([HD, 512], F32, tag="bcs",
                                        name="bcs")
                    nc.gpsimd.partition_broadcast(bcs[:], rc[:])
                    nc.vector.tensor_mul(
                        oT[p][po:po + HD, c * TQ:(c + 1) * TQ],
                        o_sb[:], bcs[:])
                    if p == 3 and po == 64:
                        # chunk c of the last pair fully normalized: its
                        # output projection becomes available as filler
                        filler.append(gen_c(c))

            # pair 3 runs its chunks largest-first: chunk 3's output
            # projection then feeds the starved small chunks as filler
            for c in (range(T // TQ) if p < 3 else (3, 2, 1, 0)):
                ntk = 4 * c + 4
                for h01 in range(2):
                    hl = 2 * p + h01
                    po = h01 * 64
                    op = osp.tile([128, 512], F32, tag="osp", name="osp")
                    for t0g in range(0, ntk, GRP):
                        ps = scp.tile([128, GRP * 512], F32, tag="scp",
                                      name="scp")
                        vss = []
                        for s in range(GRP):
                            t = t0g + s
                            vs = max(0, t - 4 * c) * 128
                            vss.append(vs)
                            nc.tensor.matmul(
                                ps[:, s * 512 + vs:(s + 1) * 512],
                                lhsT=kt[po:po + 64,
                                        t * 128:(t + 1) * 128],
                                rhs=qt[po:po + 64,
                                       c * TQ + vs:(c + 1) * TQ],
                                start=True, stop=True)
                        pt = pt_pool.tile([128, GRP * 512], DT,
                                          tag="pt", name="pt")
                        # exp over the valid (non-fully-masked) region only
                        if vss[1] == 0:
                            nc.scalar.activation(
                                pt[:, vss[0]:GRP * 512],
                                ps[:, vss[0]:GRP * 512], EXPF, scale=0.125)
                        else:
                            nc.scalar.activation(
                                pt[:, vss[0]:512],
                                ps[:, vss[0]:512], EXPF, scale=0.125)
                            nc.scalar.activation(
                                pt[:, 512 + vss[1]:1024],
                                ps[:, 512 + vss[1]:1024], EXPF, scale=0.125)
                        for s in range(GRP):
                            t = t0g + s
                            j = t - 4 * c
                            if j >= 0:  # mask the diagonal 128x128 block
                                blk = pt[:, s * 512 + j * 128:
                                         s * 512 + (j + 1) * 128]
                                nc.vector.tensor_mul(blk, blk, mask[:])
                        # early chunks have little PE work per exp group, so
                        # feed them more filler to keep the PE streaming
                        pump((3, 2, 2, 1)[c] if p < 3 else (5, 3, 3, 1)[c])
                        for s in range(GRP):
                            t = t0g + s
                            if len(pending) >= LAG:
                                emit_av(pending.pop(0))
                            pending.append((hl, c, po, op, t, ntk,
                                            pt[:, s * 512:(s + 1) * 512]))
            while pending:
                emit_av(pending.pop(0))
                # keep the PE fed while the drain ladder waits on exp
                pump(3)
        # drain remaining filler (tail of the output projection)
        while filler:
            try:
                next(filler[0])
            except StopIteration:
                filler.popleft()

    nc.compile()
    return nc


_CACHE = {}


def _get_nc(with_bias: bool):
    if with_bias not in _CACHE:
        _CACHE[with_bias] = _build(with_bias)
    return _CACHE[with_bias]


def make_in_maps(x, w_qkv, b_qkv, w_proj, with_bias):
    """Per-core input dicts (host-side shard + transpose + bf16 cast)."""
    x = np.asarray(x, dtype=np.float32)
    w_qkv = np.asarray(w_qkv, dtype=np.float32)
    b_qkv = np.asarray(b_qkv, dtype=np.float32)
    w_proj = np.asarray(w_proj, dtype=np.float32)
    cast = lambda a: np.ascontiguousarray(a).astype(ml_dtypes.bfloat16)
    in_maps = []
    for core in range(NCORES):
        b, hg = divmod(core, 2)
        # w columns ordered [q0k0|q1k1|q2k2|q3k3|v]: per head pair p, its
        # 128 q columns then its 128 k columns; the v block unchanged
        qk_cols = []
        for p in range(4):
            qk_cols.append(np.r_[hg * DS + 128 * p:hg * DS + 128 * (p + 1)])
            qk_cols.append(np.r_[D + hg * DS + 128 * p:
                                 D + hg * DS + 128 * (p + 1)])
        qk_cols.append(np.arange(2 * D + hg * DS, 2 * D + hg * DS + DS))
        cols = np.concatenate(qk_cols)
        w_s = w_qkv[:, cols]                      # [D, 3*DS]
        xTa = np.ascontiguousarray(x[b].T)        # [D, T]
        if with_bias:
            xTa = np.concatenate([xTa, np.ones((1, T), np.float32)], axis=0)
            w_s = np.concatenate([w_s, b_qkv[cols][None, :]], axis=0)
        in_maps.append({
            "xT": cast(xTa),
            "w": cast(w_s),
            "wp": cast(w_proj[hg * DS:(hg + 1) * DS, :]),
        })
    return in_maps


LAST_EXEC_TIME_NS = None


def kernel(x, w_qkv, b_qkv, w_proj, b_proj):
    global LAST_EXEC_TIME_NS
    with_bias = bool(np.any(np.asarray(b_qkv)))
    nc = _get_nc(with_bias)
    in_maps = make_in_maps(x, w_qkv, b_qkv, w_proj, with_bias)
    res = bass_utils.run_bass_kernel_spmd(
        nc, in_maps, core_ids=list(range(NCORES)))
    LAST_EXEC_TIME_NS = res.exec_time_ns
    b_proj = np.asarray(b_proj, dtype=np.float32)
    out = np.empty((B, T, D), dtype=np.float32)
    for b in range(B):
        out[b] = (np.asarray(res.results[2 * b]["out"], dtype=np.float32)
                  + np.asarray(res.results[2 * b + 1]["out"],
                               dtype=np.float32)
                  + b_proj)
    return out
